# revision 1
# baseline (speedup 1.0000x reference)
"""Masked inclusive cumsum along dim=1 on 8 TRN2 NeuronCores.

out = cumsum(where(mask, x, 0), axis=1) computed in fp32, written fp16.
Input x: (8192, 32768) fp16, mask: (8192, 32768) bool.

Sharding: rows (dim 0) split evenly across 8 cores — each row's scan is
independent (pure data parallelism, no collectives).

Per-core kernel (1024 rows x 32768 cols):
  - 8 row-tiles of 128 partitions; columns processed in chunks.
  - ACT (scalar engine) converts the u8 mask chunk to fp16 (keeps DVE free).
  - DVE tensor_tensor mult x*mask -> fp16 v (2x_1p perf mode; exact since
    mask is 0/1).
  - DVE tensor_tensor_scan: state = (v + state) [op1=bypass ignores data1],
    fp32 internal state, writes fp16 directly.
  - Cross-chunk carry: ACT copies the last scan column into a persistent
    fp32 carry tile; the next chunk's scan uses it as `initial`.
"""

import sys
from contextlib import ExitStack

import numpy as np

for _p in ("/opt/trn_rl_repo", "/opt/pypackages"):
    if _p not in sys.path:
        sys.path.insert(0, _p)

import concourse.bass as bass  # noqa: E402
import concourse.tile as tile  # noqa: E402
from concourse import bacc, mybir  # noqa: E402
from concourse.bass_utils import run_bass_kernel_spmd  # noqa: E402

ROWS, N = 8192, 32768
N_CORES = 8
ROWS_PER_CORE = ROWS // N_CORES  # 1024
P = 128
CHUNK = 8192

_BUILD_CACHE: dict = {}


def build(rows=ROWS_PER_CORE, n=N, chunk=CHUNK):
    key = (rows, n, chunk)
    if key in _BUILD_CACHE:
        return _BUILD_CACHE[key]

    assert rows % P == 0 and n % chunk == 0
    n_rt = rows // P
    n_ch = n // chunk

    nc = bacc.Bacc("TRN2", target_bir_lowering=False, debug=False)
    x_ap = nc.dram_tensor("x", (rows, n), mybir.dt.float16, kind="ExternalInput").ap()
    m_ap = nc.dram_tensor("mask", (rows, n), mybir.dt.uint8, kind="ExternalInput").ap()
    o_ap = nc.dram_tensor("out", (rows, n), mybir.dt.float16, kind="ExternalOutput").ap()

    with tile.TileContext(nc) as tc, ExitStack() as ctx:
        xp = ctx.enter_context(tc.tile_pool(name="xp", bufs=3))
        mp = ctx.enter_context(tc.tile_pool(name="mp", bufs=3))
        fp = ctx.enter_context(tc.tile_pool(name="fp", bufs=2))
        vp = ctx.enter_context(tc.tile_pool(name="vp", bufs=2))
        op = ctx.enter_context(tc.tile_pool(name="op", bufs=3))
        cp = ctx.enter_context(tc.tile_pool(name="cp", bufs=1))

        # Persistent per-row-tile carry (fp32: scan `initial` scalars are fp32).
        carry = cp.tile([P, n_rt], mybir.dt.float32)

        for c in range(n_ch):
            c0 = c * chunk
            for rt in range(n_rt):
                r0 = rt * P
                xt = xp.tile([P, chunk], mybir.dt.float16)
                nc.sync.dma_start(xt[:], x_ap[r0 : r0 + P, c0 : c0 + chunk])
                mt = mp.tile([P, chunk], mybir.dt.uint8)
                nc.sync.dma_start(mt[:], m_ap[r0 : r0 + P, c0 : c0 + chunk])

                m16 = fp.tile([P, chunk], mybir.dt.float16)
                nc.scalar.copy(m16[:], mt[:])

                vt = vp.tile([P, chunk], mybir.dt.float16)
                nc.vector.tensor_mul(vt[:], xt[:], m16[:])

                ot = op.tile([P, chunk], mybir.dt.float16)
                init = 0.0 if c == 0 else carry[:, rt : rt + 1]
                nc.vector.tensor_tensor_scan(
                    ot[:],
                    vt[:],
                    vt[:],
                    init,
                    op0=mybir.AluOpType.add,
                    op1=mybir.AluOpType.bypass,
                )
                if c + 1 < n_ch:
                    nc.scalar.copy(carry[:, rt : rt + 1], ot[:, chunk - 1 : chunk])

                nc.gpsimd.dma_start(o_ap[r0 : r0 + P, c0 : c0 + chunk], ot[:])

    nc.compile()
    _BUILD_CACHE[key] = nc
    return nc


def _in_maps(x, mask):
    x = np.asarray(x)
    mask = np.asarray(mask)
    if mask.dtype == np.bool_:
        m8 = mask.view(np.uint8)
    else:
        m8 = mask.astype(np.uint8)
    if x.dtype != np.float16:
        x = x.astype(np.float16)
    rpc = x.shape[0] // N_CORES
    return [
        {
            "x": np.ascontiguousarray(x[i * rpc : (i + 1) * rpc]),
            "mask": np.ascontiguousarray(m8[i * rpc : (i + 1) * rpc]),
        }
        for i in range(N_CORES)
    ], rpc


def run(x, mask, trace=False, **trace_kwargs):
    """Returns (out, BassKernelResults)."""
    in_maps, rpc = _in_maps(x, mask)
    nc = build(rows=rpc, n=np.asarray(x).shape[1])
    res = run_bass_kernel_spmd(
        nc, in_maps, core_ids=list(range(N_CORES)), trace=trace, **trace_kwargs
    )
    out = np.concatenate([res.results[i]["out"] for i in range(N_CORES)], axis=0)
    return out.astype(np.float16), res


def kernel(x, mask):
    out, _ = run(x, mask, trace=False)
    return out


# revision 3
# speedup vs baseline: 1.4818x; 1.4818x over previous
"""Masked inclusive cumsum along dim=1 on 8 TRN2 NeuronCores.

out = cumsum(where(mask, x, 0), axis=1) computed in fp32, written fp16.
Input x: (8192, 32768) fp16, mask: (8192, 32768) bool.

Sharding: rows (dim 0) split evenly across 8 cores — each row's scan is
independent (pure data parallelism, no collectives).

Per-core kernel (1024 rows x 32768 cols), per [128, CHUNK] tile:
  - One fused custom-DVE op does the whole thing:
      body = scan(ADD, Src0 * Src1, init=C0)
    i.e. out[p,k] = carry[p] + sum_{j<=k} x[p,j]*mask[p,j], fp32 internal
    state, fp16 output, mask read directly as u8. The Spec-DSL scan uses
    same-stage ALU feedback => 1 elem/cycle/lane (the stock
    TensorTensorScanArith runs at 2 cyc/elem and needs a separate masked
    multiply + mask-dtype conversion).
  - ACT copies the last output column into an fp32 carry tile for the next
    chunk's init (the only cross-chunk dependency).
DVE busy ~278 us/core; HBM traffic 160 MiB/core (~450 us) is the roofline.
"""

import sys
from contextlib import ExitStack

import numpy as np

for _p in ("/opt/trn_rl_repo", "/opt/pypackages"):
    if _p not in sys.path:
        sys.path.insert(0, _p)

import concourse.bass as bass  # noqa: E402
import concourse.tile as tile  # noqa: E402
from concourse import bacc, mybir  # noqa: E402
from concourse.bass_utils import run_bass_kernel_spmd  # noqa: E402

ROWS, N = 8192, 32768
N_CORES = 8
ROWS_PER_CORE = ROWS // N_CORES  # 1024
P = 128
CHUNK = 8192

_BUILD_CACHE: dict = {}


def _masked_cumsum_ref(in0, in1, c0, c1, c2):
    """CoreSim reference for MASKED_CUMSUM_ANT: c0 + cumsum(in0*in1, fp32)."""
    v = in0.astype(np.float32) * np.asarray(in1).astype(np.float32)
    shp = v.shape
    cs = np.cumsum(v.reshape(shp[0], -1), axis=-1, dtype=np.float32)
    cs = cs + (c0.reshape(-1, 1) if isinstance(c0, np.ndarray) else c0)
    return cs.reshape(shp)


def _register_custom_op():
    """Register the fused masked-cumsum DVE op with concourse's custom-op
    registry (op table, sim reference, sub-opcode row) for this process."""
    from concourse import dve_ops
    from concourse.dve_spec import C0, AluOp, Spec, Src0, Src1, lower, scan
    from concourse.dve_uop import DveOpSpec

    name = "MASKED_CUMSUM_ANT"
    for o in dve_ops.OPS:
        if o.name == name:
            return o
    spec = Spec(
        body=scan(AluOp.ADD, Src0 * Src1, init=C0),
        reference=_masked_cumsum_ref,
    )
    opcode = dve_ops._CUSTOM_DVE_ROW_BASE + len(dve_ops.OPS)
    uops = lower(spec, ver="v3")
    sha = DveOpSpec(name=name, opcode=opcode, uops=uops, rd1_en=True).sha("v3")
    op = dve_ops.DveOp(name, spec, subdim=False, uops_sha={"v3": sha})
    dve_ops.OPS.append(op)
    dve_ops.CUSTOM_DVE_SPECS[name] = spec
    dve_ops._SUB_OPCODE_FOR_NAME[name] = opcode
    return op


MASKED_CUMSUM_ANT = _register_custom_op()


def build(rows=ROWS_PER_CORE, n=N, chunk=CHUNK):
    key = (rows, n, chunk)
    if key in _BUILD_CACHE:
        return _BUILD_CACHE[key]

    assert rows % P == 0 and n % chunk == 0
    n_rt = rows // P
    n_ch = n // chunk

    nc = bacc.Bacc("TRN2", target_bir_lowering=False, debug=False)
    x_ap = nc.dram_tensor("x", (rows, n), mybir.dt.float16, kind="ExternalInput").ap()
    m_ap = nc.dram_tensor("mask", (rows, n), mybir.dt.uint8, kind="ExternalInput").ap()
    o_ap = nc.dram_tensor("out", (rows, n), mybir.dt.float16, kind="ExternalOutput").ap()

    with tile.TileContext(nc) as tc, ExitStack() as ctx:
        xp = ctx.enter_context(tc.tile_pool(name="xp", bufs=4))
        mp = ctx.enter_context(tc.tile_pool(name="mp", bufs=4))
        op_ = ctx.enter_context(tc.tile_pool(name="op", bufs=4))
        cp = ctx.enter_context(tc.tile_pool(name="cp", bufs=3 * n_rt))

        carries: dict = {}
        for c in range(n_ch):
            c0 = c * chunk
            for rt in range(n_rt):
                r0 = rt * P
                xt = xp.tile([P, chunk], mybir.dt.float16)
                nc.sync.dma_start(xt[:], x_ap[r0 : r0 + P, c0 : c0 + chunk])
                mt = mp.tile([P, chunk], mybir.dt.uint8)
                nc.sync.dma_start(mt[:], m_ap[r0 : r0 + P, c0 : c0 + chunk])

                ot = op_.tile([P, chunk], mybir.dt.float16)
                init = 0.0 if c == 0 else carries[rt][:]
                nc.vector._custom_dve(
                    MASKED_CUMSUM_ANT, out=ot[:], in0=xt[:], in1=mt[:], s0=init
                )
                if c + 1 < n_ch:
                    cnew = cp.tile([P, 1], mybir.dt.float32)
                    nc.scalar.copy(cnew[:], ot[:, chunk - 1 : chunk])
                    carries[rt] = cnew

                nc.gpsimd.dma_start(o_ap[r0 : r0 + P, c0 : c0 + chunk], ot[:])

    nc.compile()
    _BUILD_CACHE[key] = nc
    return nc


def _in_maps(x, mask):
    x = np.asarray(x)
    mask = np.asarray(mask)
    if mask.dtype == np.bool_:
        m8 = mask.view(np.uint8)
    else:
        m8 = mask.astype(np.uint8)
    if x.dtype != np.float16:
        x = x.astype(np.float16)
    rpc = x.shape[0] // N_CORES
    return [
        {
            "x": np.ascontiguousarray(x[i * rpc : (i + 1) * rpc]),
            "mask": np.ascontiguousarray(m8[i * rpc : (i + 1) * rpc]),
        }
        for i in range(N_CORES)
    ], rpc


def run(x, mask, trace=False, **trace_kwargs):
    """Returns (out, BassKernelResults)."""
    in_maps, rpc = _in_maps(x, mask)
    nc = build(rows=rpc, n=np.asarray(x).shape[1])
    res = run_bass_kernel_spmd(
        nc, in_maps, core_ids=list(range(N_CORES)), trace=trace, **trace_kwargs
    )
    out = np.concatenate([res.results[i]["out"] for i in range(N_CORES)], axis=0)
    return out.astype(np.float16), res


def kernel(x, mask):
    out, _ = run(x, mask, trace=False)
    return out


# revision 9
# speedup vs baseline: 1.7564x; 1.1853x over previous
"""Masked inclusive cumsum along dim=1 on 8 TRN2 NeuronCores.

out = cumsum(where(mask, x, 0), axis=1) computed in fp32, written fp16.
Input x: (8192, 32768) fp16, mask: (8192, 32768) bool.

Sharding: rows (dim 0) split evenly across 8 cores — each row's scan is
independent (pure data parallelism, no collectives).

Per-core kernel (1024 rows x 32768 cols), per [128, CHUNK] tile:
  - One fused custom-DVE op does the whole thing:
      body = scan(ADD, Src0 * Src1, init=C0)
    i.e. out[p,k] = carry[p] + sum_{j<=k} x[p,j]*mask[p,j], fp32 internal
    state, fp16 output, mask read directly as u8. The Spec-DSL scan uses
    same-stage ALU feedback => 1 elem/cycle/lane (the stock
    TensorTensorScanArith runs at 2 cyc/elem and needs a separate masked
    multiply + mask-dtype conversion).
  - ACT copies the last output column into an fp32 carry tile for the next
    chunk's init (the only cross-chunk dependency).
DVE busy ~278 us/core; HBM traffic 160 MiB/core (~450 us) is the roofline.
"""

import sys
from contextlib import ExitStack

import numpy as np

for _p in ("/opt/trn_rl_repo", "/opt/pypackages"):
    if _p not in sys.path:
        sys.path.insert(0, _p)

import concourse.bass as bass  # noqa: E402
import concourse.tile as tile  # noqa: E402
from concourse import bacc, mybir  # noqa: E402
from concourse.bass_utils import run_bass_kernel_spmd  # noqa: E402

ROWS, N = 8192, 32768
N_CORES = 8
ROWS_PER_CORE = ROWS // N_CORES  # 1024
P = 128
CHUNK = 8192

_BUILD_CACHE: dict = {}


def _masked_cumsum_ref(in0, in1, c0, c1, c2):
    """CoreSim reference for MASKED_CUMSUM_ANT: c0 + cumsum(in0*in1, fp32)."""
    v = in0.astype(np.float32) * np.asarray(in1).astype(np.float32)
    shp = v.shape
    cs = np.cumsum(v.reshape(shp[0], -1), axis=-1, dtype=np.float32)
    cs = cs + (c0.reshape(-1, 1) if isinstance(c0, np.ndarray) else c0)
    return cs.reshape(shp)


def _register_custom_op():
    """Register the fused masked-cumsum DVE op with concourse's custom-op
    registry (op table, sim reference, sub-opcode row) for this process."""
    from concourse import dve_ops
    from concourse.dve_spec import C0, AluOp, Spec, Src0, Src1, lower, scan
    from concourse.dve_uop import DveOpSpec

    name = "MASKED_CUMSUM_ANT"
    for o in dve_ops.OPS:
        if o.name == name:
            return o
    spec = Spec(
        body=scan(AluOp.ADD, Src0 * Src1, init=C0),
        reference=_masked_cumsum_ref,
    )
    opcode = dve_ops._CUSTOM_DVE_ROW_BASE + len(dve_ops.OPS)
    uops = lower(spec, ver="v3")
    sha = DveOpSpec(name=name, opcode=opcode, uops=uops, rd1_en=True).sha("v3")
    op = dve_ops.DveOp(name, spec, subdim=False, uops_sha={"v3": sha})
    dve_ops.OPS.append(op)
    dve_ops.CUSTOM_DVE_SPECS[name] = spec
    dve_ops._SUB_OPCODE_FOR_NAME[name] = opcode
    return op


MASKED_CUMSUM_ANT = _register_custom_op()


def build(
    rows=ROWS_PER_CORE,
    n=N,
    chunk=CHUNK,
    bufs=(5, 4, 4),
    out_eng="gpsimd",
    carry_eng="scalar",
):
    key = (rows, n, chunk, bufs, out_eng, carry_eng)
    if key in _BUILD_CACHE:
        return _BUILD_CACHE[key]

    assert rows % P == 0 and n % chunk == 0
    n_rt = rows // P
    n_ch = n // chunk

    nc = bacc.Bacc("TRN2", target_bir_lowering=False, debug=False)
    x_ap = nc.dram_tensor("x", (rows, n), mybir.dt.float16, kind="ExternalInput").ap()
    m_ap = nc.dram_tensor("mask", (rows, n), mybir.dt.uint8, kind="ExternalInput").ap()
    o_ap = nc.dram_tensor("out", (rows, n), mybir.dt.float16, kind="ExternalOutput").ap()

    with tile.TileContext(nc) as tc, ExitStack() as ctx:
        xp = ctx.enter_context(tc.tile_pool(name="xp", bufs=bufs[0]))
        mp = ctx.enter_context(tc.tile_pool(name="mp", bufs=bufs[1]))
        op_ = ctx.enter_context(tc.tile_pool(name="op", bufs=bufs[2]))
        cp = ctx.enter_context(tc.tile_pool(name="cp", bufs=3 * n_rt))

        carries: dict = {}
        for c in range(n_ch):
            c0 = c * chunk
            for rt in range(n_rt):
                r0 = rt * P
                xt = xp.tile([P, chunk], mybir.dt.float16)
                nc.sync.dma_start(xt[:], x_ap[r0 : r0 + P, c0 : c0 + chunk])
                mt = mp.tile([P, chunk], mybir.dt.uint8)
                nc.sync.dma_start(mt[:], m_ap[r0 : r0 + P, c0 : c0 + chunk])

                ot = op_.tile([P, chunk], mybir.dt.float16)
                init = 0.0 if c == 0 else carries[rt][:]
                nc.vector._custom_dve(
                    MASKED_CUMSUM_ANT, out=ot[:], in0=xt[:], in1=mt[:], s0=init
                )
                if c + 1 < n_ch:
                    cnew = cp.tile([P, 1], mybir.dt.float32)
                    if carry_eng == "scalar":
                        nc.scalar.copy(cnew[:], ot[:, chunk - 1 : chunk])
                    else:
                        getattr(nc, carry_eng).tensor_copy(
                            cnew[:], ot[:, chunk - 1 : chunk]
                        )
                    carries[rt] = cnew

                getattr(nc, out_eng).dma_start(
                    o_ap[r0 : r0 + P, c0 : c0 + chunk], ot[:]
                )

    nc.compile()
    _BUILD_CACHE[key] = nc
    return nc


def _in_maps(x, mask):
    x = np.asarray(x)
    mask = np.asarray(mask)
    if mask.dtype == np.bool_:
        m8 = mask.view(np.uint8)
    else:
        m8 = mask.astype(np.uint8)
    if x.dtype != np.float16:
        x = x.astype(np.float16)
    rpc = x.shape[0] // N_CORES
    return [
        {
            "x": np.ascontiguousarray(x[i * rpc : (i + 1) * rpc]),
            "mask": np.ascontiguousarray(m8[i * rpc : (i + 1) * rpc]),
        }
        for i in range(N_CORES)
    ], rpc


def run(x, mask, trace=False, **trace_kwargs):
    """Returns (out, BassKernelResults)."""
    in_maps, rpc = _in_maps(x, mask)
    nc = build(rows=rpc, n=np.asarray(x).shape[1])
    res = run_bass_kernel_spmd(
        nc, in_maps, core_ids=list(range(N_CORES)), trace=trace, **trace_kwargs
    )
    out = np.concatenate([res.results[i]["out"] for i in range(N_CORES)], axis=0)
    return out.astype(np.float16), res


def kernel(x, mask):
    out, _ = run(x, mask, trace=False)
    return out
